# revision 22
# baseline (speedup 1.0000x reference)
"""GNN message-passing (Convolve) kernel for Trainium2, 8 NeuronCores.

Reference computation (B=8, N=8192, C=256, H=256, O=256, K=64):
    g   = embeddings[:, neighbor_set, :]                     # [B, K, C]
    h   = leaky_relu(g @ Qw + Qb)                            # [B, K, H]
    w   = weights[neighbor_set, node_id]                     # [K]
    s   = sum_k h * w / (sum_k w + eps)                      # [B, H]
    z   = concat(embeddings[:, node_id, :], s)               # [B, C+H]
    o   = leaky_relu(z @ Ww + Wb)                            # [B, O]
    out = o / (||o||_2 + eps)                                # [B, O]

Sharding: data-parallel over the batch axis — core b handles batch b.
The host performs all *indexing/layout* work (neighbor gather, transpose,
bf16 cast, weight-column extraction); every FLOP of the reference
computation (both matmuls, the weighted sum, the activations, the L2
normalization) runs on device.

Scale trick: leaky_relu is positively homogeneous and the final L2
normalize is scale-invariant, so instead of x = e_node@W_top +
(h^T w / den)@W_bot we compute x' = den*x = (den*e_node)@W_top +
(h^T w)@W_bot — the reciprocal of den disappears from the kernel.

Per-core device inputs (bf16, >=512B per partition line where it counts):
    gt  [128, 256]:  cols 0:64 = g[:, 0:128].T, cols 64:128 = g[:,128:256].T,
                     col 128 = w as a column (K=64 partitions),
                     col 132/136 = node embedding halves
    qwt [128, 512]:  [Qw[0:128, :] | Qw[128:256, :]]
    wwt [128, 1024]: [Ww[0:128,:] | Ww[128:256,:] | Ww[256:384,:] | Ww[384:512,:]]

Device dataflow: the three input DMAs issue in parallel on the sync /
vector / scalar engine queues.  PE queue: den broadcast (ones-matmul),
4 h-matmuls, 2 s-matmuls, then the 4 x-matmuls (node parts first) — s
before x so the in-order PE queue never head-blocks on the (largest,
latest) wwt DMA.  All leaky-relus run on DVE as one scalar_tensor_tensor
(max(0.3x, x)) so no Prelu ACT table is loaded; the lone Scalar-engine
op is the final [1,1] Sqrt.  Epilogue: leaky (DVE), square+norm2 in one
op, Sqrt (ACT), divide-by-norm (DVE), contiguous 1KB out DMA.
"""

import functools

import numpy as np

import concourse.bacc as bacc
import concourse.bass as bass
import concourse.mybir as mybir
import concourse.tile as tile
from concourse.bass_utils import run_bass_kernel_spmd

B, N, C, H, O, K = 8, 8192, 256, 256, 256, 64
ALPHA = 0.3
F32 = mybir.dt.float32
BF16 = mybir.dt.bfloat16
N_CORES = 8
MULT = mybir.AluOpType.mult
ADD = mybir.AluOpType.add
SUB = mybir.AluOpType.subtract
MAX = mybir.AluOpType.max
BYP = mybir.AluOpType.bypass
ASR = mybir.AluOpType.arith_shift_right
I32 = mybir.dt.int32
AF = mybir.ActivationFunctionType

# all-DVE fast inverse sqrt (Quake seed + 1 Newton step, <=0.17% rel err)
# instead of Scalar-engine Sqrt + DVE reciprocal: ~6 same-engine [1,1] ops,
# no cross-engine hops, no Sqrt ACT table.
USE_DVE_RSQRT = True


def _build_program(has_qb: bool, has_wb: bool) -> bass.Bass:
    nc = bacc.Bacc(None, target_bir_lowering=False, debug=False)

    gtq_d = nc.dram_tensor("gtq", [128, 768], BF16, kind="ExternalInput")
    wwt_d = nc.dram_tensor("wwt", [128, 1024], BF16, kind="ExternalInput")
    if has_qb:
        qb_d = nc.dram_tensor("qb", [1, H], BF16, kind="ExternalInput")
    if has_wb:
        wb_d = nc.dram_tensor("wb", [1, O], F32, kind="ExternalInput")
    out_d = nc.dram_tensor("out", [1, O], F32, kind="ExternalOutput")

    with tile.TileContext(nc) as tc:
        with (
            tc.tile_pool(name="sb", bufs=1) as sb,
            tc.tile_pool(name="ps", bufs=1, space="PSUM") as ps,
        ):
            # ---- input DMAs. gt+qwt (both needed to start the h chain)
            # ride ONE 192KB DMA on the sync queue — the per-DMA doorbell
            # latency is ~0.9us and separate queues serialize their
            # transfers anyway, so one merged DMA lands both ~0.7us sooner.
            # wwt (needed last) issues in parallel on the scalar queue. ----
            gtq = sb.tile([128, 768], BF16)
            nc.sync.dma_start(out=gtq[:], in_=gtq_d[:])
            gt = gtq  # cols 0:256 = gt
            qwt_off = 256  # cols 256:768 = qwt
            wwt = sb.tile([128, 1024], BF16)
            nc.scalar.dma_start(out=wwt[:], in_=wwt_d[:])
            if has_qb:
                qb = sb.tile([1, H], BF16)
                nc.gpsimd.dma_start(out=qb[:], in_=qb_d[:])
            if has_wb:
                wb = sb.tile([1, O], F32)
                nc.gpsimd.dma_start(out=wb[:], in_=wb_d[:])

            # ---- constants (no DMA deps) ----
            ones_m = sb.tile([K, 128], BF16)
            nc.gpsimd.memset(ones_m[:], 1.0)
            if has_qb:
                onesk = sb.tile([1, K], BF16)
                nc.gpsimd.memset(onesk[:], 1.0)

            if USE_DVE_RSQRT:
                # magic-constant tile for the rsqrt seed
                rsq_c = sb.tile([1, 1], I32)
                nc.vector.memset(rsq_c[:], 0x5F3759DF)
            else:
                # ---- warm the Sqrt ACT table: the compiler inserts each ACT
                # table load right before the first ACT that uses it, in
                # queue order. Without this warm the Sqrt table (1283ns load)
                # lands on the critical path right before the final [1,1]
                # sqrt. The warm input comes from a DVE memset so it has no
                # DMA dependency. ----
                warm_in = sb.tile([1, 1], F32)
                nc.vector.memset(warm_in[:], 1.0)
                warm_t = sb.tile([1, 1], F32)
                nc.scalar.activation(out=warm_t[:], in_=warm_in[:], func=AF.Sqrt)

            # ---- den = sum(w) broadcast across 128 partitions via
            # ones-matrix matmul (ones[K,128].T @ w = sum(w) per partition) ----
            den_bp = ps.tile([128, 1], F32, tag="rb")
            nc.tensor.matmul(
                out=den_bp[:], lhsT=ones_m[:], rhs=gt[0:K, 128:129],
                start=True, stop=True, skip_group_check=True,
            )

            # ---- h = leaky(gT.T @ Qw (+Qb)), split by h-column halves.
            # Separate PSUM tiles per half so each accumulation group gets
            # its own PSUM bank. ----
            h_ps = []
            for j in range(2):
                h_half = ps.tile([K, 128], F32, tag=f"h{j}", name=f"h_half{j}")
                h_ps.append(h_half)
            for j in range(2):
                nc.tensor.matmul(
                    out=h_ps[j][:], lhsT=gt[:, 0:64],
                    rhs=gtq[:, qwt_off + 128 * j : qwt_off + 128 * (j + 1)],
                    start=True, stop=False, skip_group_check=True,
                )
                nc.tensor.matmul(
                    out=h_ps[j][:], lhsT=gt[:, 64:128],
                    rhs=gtq[:, qwt_off + 256 + 128 * j : qwt_off + 384 + 128 * j],
                    start=False, stop=not has_qb, skip_group_check=True,
                )
                if has_qb:
                    nc.tensor.matmul(
                        out=h_ps[j][:], lhsT=onesk[:],
                        rhs=qb[:, 128 * j : 128 * (j + 1)],
                        start=False, stop=True, skip_group_check=True,
                    )

            # ---- DVE: e_scaled = e_node * den (per-partition scalar from
            # the den broadcast), then leaky-relu h halves as max(0.3x, x) ----
            e_s = sb.tile([128, 2], BF16)
            nc.vector.tensor_scalar_mul(e_s[:, 0:1], gt[:, 132:133], den_bp[:])
            nc.vector.tensor_scalar_mul(e_s[:, 1:2], gt[:, 136:137], den_bp[:])
            h_l = sb.tile([K, H], BF16)
            for j in range(2):
                nc.scalar.activation(
                    out=h_l[:, 128 * j : 128 * (j + 1)], in_=h_ps[j][:],
                    func=AF.Prelu, alpha=ALPHA,
                )

            # ---- s chunks on PE back-to-back, then the x group:
            # node parts (den-scaled) first, s parts last. High priority so
            # the scheduler runs them as soon as their leaky half is done
            # (z1 gates the last x matmuls -> the whole epilogue). ----
            s_ps = []
            for j in range(2):
                s_p = ps.tile([128, 1], F32, tag=f"s{j}", name=f"s_p{j}")
                nc.tensor.matmul(
                    out=s_p[:], lhsT=h_l[:, 128 * j : 128 * (j + 1)],
                    rhs=gt[0:K, 128:129], start=True, stop=True,
                    skip_group_check=True,
                )
                s_ps.append(s_p)

            # PSUM -> SBUF copies of the s chunks (plain, no scaling needed)
            zs = []
            for j in range(2):
                z = sb.tile([128, 1], BF16, tag=f"z{j}", name=f"z{j}")
                nc.vector.tensor_scalar_mul(z[:], s_ps[j][:], 1.0)
                zs.append(z)

            # ---- x in two [1,128] column halves: 8 cheap free=128 matmuls
            # instead of 4 free=256 ones, and the first half's epilogue
            # (leaky + square-sum) overlaps the second half's matmuls ----
            if has_wb:
                den_sb = sb.tile([1, 1], F32)
                nc.vector.tensor_scalar_mul(den_sb[:], den_bp[0:1, :], 1.0)
            o2 = sb.tile([1, O], F32)
            n2s = []
            for m in range(2):
                x_p = ps.tile([1, 128], F32, tag=f"x{m}", name=f"x{m}")
                for j in range(2):
                    nc.tensor.matmul(
                        out=x_p[:], lhsT=e_s[:, j : j + 1],
                        rhs=wwt[:, 256 * j + 128 * m : 256 * j + 128 * (m + 1)],
                        start=(j == 0), stop=False, skip_group_check=True,
                    )
                for j in range(2):
                    nc.tensor.matmul(
                        out=x_p[:], lhsT=zs[j][:],
                        rhs=wwt[:, 512 + 256 * j + 128 * m : 512 + 256 * j + 128 * (m + 1)],
                        start=False, stop=(j == 1), skip_group_check=True,
                    )
                if has_wb:
                    # x2 = wb * den + x  (bias must also be den-scaled)
                    x2 = sb.tile([1, 128], F32, name=f"x2_{m}")
                    nc.vector.scalar_tensor_tensor(
                        out=x2[:], in0=wb[:, 128 * m : 128 * (m + 1)],
                        scalar=den_sb[:], in1=x_p[:], op0=MULT, op1=ADD,
                    )
                    xsrc = x2
                else:
                    xsrc = x_p
                o2h = o2[:, 128 * m : 128 * (m + 1)]
                nc.scalar.activation(
                    out=o2h, in_=xsrc[:], func=AF.Prelu, alpha=ALPHA
                )
                sq = sb.tile([1, 128], F32, name=f"sq{m}")
                n2 = sb.tile([1, 1], F32, name=f"n2_{m}")
                nc.vector.scalar_tensor_tensor(
                    out=sq[:], in0=o2h, scalar=1.0, in1=o2h,
                    op0=MULT, op1=MULT, accum_out=n2[:],
                )
                n2s.append(n2)

            res = sb.tile([1, O], F32)
            if USE_DVE_RSQRT:
                # rc2 = rsqrt(n2a + n2b), entirely on DVE (no engine hops):
                # seed y0 = bitcast(C - (bitcast(n2) >> 1)), one Newton step
                # y1 = y0 * (1.5 - 0.5 * n2 * y0^2).
                n2 = sb.tile([1, 1], F32)
                nc.vector.scalar_tensor_tensor(
                    out=n2[:], in0=n2s[0][:], scalar=1.0, in1=n2s[1][:],
                    op0=MULT, op1=ADD,
                )
                t1 = sb.tile([1, 1], I32)
                nc.vector.tensor_scalar(
                    out=t1[:], in0=n2[:].bitcast(I32), scalar1=1, scalar2=None,
                    op0=ASR,
                )
                y0 = sb.tile([1, 1], F32)
                nc.vector.scalar_tensor_tensor(
                    out=y0[:].bitcast(I32), in0=rsq_c[:], scalar=0, in1=t1[:],
                    op0=BYP, op1=SUB,
                )
                u = sb.tile([1, 1], F32)
                nc.vector.scalar_tensor_tensor(
                    out=u[:], in0=y0[:], scalar=0.0, in1=y0[:],
                    op0=BYP, op1=MULT,
                )
                v = sb.tile([1, 1], F32)
                nc.vector.tensor_scalar(
                    out=v[:], in0=u[:], scalar1=n2[:], scalar2=-0.5,
                    op0=MULT, op1=MULT,
                )
                rc2 = sb.tile([1, 1], F32)
                nc.vector.scalar_tensor_tensor(
                    out=rc2[:], in0=v[:], scalar=1.5, in1=y0[:],
                    op0=ADD, op1=MULT,
                )
            else:
                # nrm = sqrt(n2a + n2b) in one ACT op via the bias input
                nrm = sb.tile([1, 1], F32)
                nc.scalar.activation(
                    out=nrm[:], in_=n2s[1][:], func=AF.Sqrt, bias=n2s[0][:]
                )
                rc2 = sb.tile([1, 1], F32)
                nc.vector.reciprocal(rc2[:], nrm[:])
            nc.vector.tensor_scalar_mul(res[:], o2[:], rc2[:])

            nc.sync.dma_start(out=out_d[:], in_=res[:], single_packet=True)

    nc.finalize()
    return nc


@functools.lru_cache(maxsize=4)
def _program(has_qb: bool, has_wb: bool) -> bass.Bass:
    return _build_program(has_qb, has_wb)


def kernel(
    embeddings: np.ndarray,
    weights: np.ndarray,
    Qw: np.ndarray,
    Qb: np.ndarray,
    Ww: np.ndarray,
    Wb: np.ndarray,
    neighbor_set: np.ndarray,
    node_id,
    _trace: bool = False,
):
    import ml_dtypes

    bf16 = ml_dtypes.bfloat16
    node_id = int(np.asarray(node_id))
    nbr = np.asarray(neighbor_set).astype(np.int64).reshape(K)
    emb = np.asarray(embeddings, dtype=np.float32)
    qb_full = np.asarray(Qb, dtype=np.float32).reshape(H)
    wb_full = np.asarray(Wb, dtype=np.float32).reshape(O)
    has_qb = bool(np.any(qb_full))
    has_wb = bool(np.any(wb_full))

    # shared (core-independent) weight tiles
    qw_np = np.asarray(Qw, dtype=np.float32)
    ww_np = np.asarray(Ww, dtype=np.float32)
    qwt = np.concatenate([qw_np[0:128, :], qw_np[128:256, :]], axis=1).astype(bf16)
    wwt = np.concatenate(
        [ww_np[128 * j : 128 * (j + 1), :] for j in range(4)], axis=1
    ).astype(bf16)
    wcol = np.asarray(weights[nbr, node_id], dtype=np.float32)  # [K]

    nc = _program(has_qb, has_wb)
    in_maps = []
    for b in range(N_CORES):
        g = emb[b, nbr, :]  # [K, C]
        e_node = emb[b, node_id, :]  # [C]
        gtq = np.zeros((128, 768), dtype=bf16)
        gt = np.zeros((128, 256), dtype=np.float32)
        gt[:, 0:64] = g[:, 0:128].T
        gt[:, 64:128] = g[:, 128:256].T
        gt[0:K, 128] = wcol
        gt[:, 132] = e_node[0:128]
        gt[:, 136] = e_node[128:256]
        gtq[:, 0:256] = gt.astype(bf16)
        gtq[:, 256:768] = qwt
        m = {"gtq": gtq, "wwt": wwt}
        if has_qb:
            m["qb"] = qb_full.reshape(1, H).astype(bf16)
        if has_wb:
            m["wb"] = np.ascontiguousarray(wb_full.reshape(1, O))
        in_maps.append(m)

    r = run_bass_kernel_spmd(nc, in_maps, list(range(N_CORES)), trace=_trace)
    out = np.stack([r.results[b]["out"][0] for b in range(N_CORES)], axis=0)
    if _trace:
        return out, r
    return out


# revision 23
# speedup vs baseline: 1.0249x; 1.0249x over previous
"""GNN message-passing (Convolve) kernel for Trainium2, 8 NeuronCores.

Reference computation (B=8, N=8192, C=256, H=256, O=256, K=64):
    g   = embeddings[:, neighbor_set, :]                     # [B, K, C]
    h   = leaky_relu(g @ Qw + Qb)                            # [B, K, H]
    w   = weights[neighbor_set, node_id]                     # [K]
    s   = sum_k h * w / (sum_k w + eps)                      # [B, H]
    z   = concat(embeddings[:, node_id, :], s)               # [B, C+H]
    o   = leaky_relu(z @ Ww + Wb)                            # [B, O]
    out = o / (||o||_2 + eps)                                # [B, O]

Sharding: data-parallel over the batch axis — core b handles batch b.
The host performs all *indexing/layout* work (neighbor gather, transpose,
bf16 cast, weight-column extraction); every FLOP of the reference
computation (both matmuls, the weighted sum, the activations, the L2
normalization) runs on device.

Scale trick: leaky_relu is positively homogeneous and the final L2
normalize is scale-invariant, so instead of x = e_node@W_top +
(h^T w / den)@W_bot we compute x' = den*x = (den*e_node)@W_top +
(h^T w)@W_bot — the reciprocal of den disappears from the kernel.

Per-core device inputs (bf16, >=512B per partition line where it counts):
    gt  [128, 256]:  cols 0:64 = g[:, 0:128].T, cols 64:128 = g[:,128:256].T,
                     col 128 = w as a column (K=64 partitions),
                     col 132/136 = node embedding halves
    qwt [128, 512]:  [Qw[0:128, :] | Qw[128:256, :]]
    wwt [128, 1024]: [Ww[0:128,:] | Ww[128:256,:] | Ww[256:384,:] | Ww[384:512,:]]

Device dataflow: the three input DMAs issue in parallel on the sync /
vector / scalar engine queues.  PE queue: den broadcast (ones-matmul),
4 h-matmuls, 2 s-matmuls, then the 4 x-matmuls (node parts first) — s
before x so the in-order PE queue never head-blocks on the (largest,
latest) wwt DMA.  All leaky-relus run on DVE as one scalar_tensor_tensor
(max(0.3x, x)) so no Prelu ACT table is loaded; the lone Scalar-engine
op is the final [1,1] Sqrt.  Epilogue: leaky (DVE), square+norm2 in one
op, Sqrt (ACT), divide-by-norm (DVE), contiguous 1KB out DMA.
"""

import functools

import numpy as np

import concourse.bacc as bacc
import concourse.bass as bass
import concourse.mybir as mybir
import concourse.tile as tile
from concourse.bass_utils import run_bass_kernel_spmd

B, N, C, H, O, K = 8, 8192, 256, 256, 256, 64
ALPHA = 0.3
F32 = mybir.dt.float32
BF16 = mybir.dt.bfloat16
N_CORES = 8
MULT = mybir.AluOpType.mult
ADD = mybir.AluOpType.add
SUB = mybir.AluOpType.subtract
MAX = mybir.AluOpType.max
BYP = mybir.AluOpType.bypass
ASR = mybir.AluOpType.arith_shift_right
I32 = mybir.dt.int32
AF = mybir.ActivationFunctionType

# all-DVE fast inverse sqrt (Quake seed + 1 Newton step, <=0.17% rel err)
# instead of Scalar-engine Sqrt + DVE reciprocal: ~6 same-engine [1,1] ops,
# no cross-engine hops, no Sqrt ACT table.
USE_DVE_RSQRT = False


def _build_program(has_qb: bool, has_wb: bool) -> bass.Bass:
    nc = bacc.Bacc(None, target_bir_lowering=False, debug=False)

    gtq_d = nc.dram_tensor("gtq", [128, 768], BF16, kind="ExternalInput")
    wwt_d = nc.dram_tensor("wwt", [128, 1024], BF16, kind="ExternalInput")
    if has_qb:
        qb_d = nc.dram_tensor("qb", [1, H], BF16, kind="ExternalInput")
    if has_wb:
        wb_d = nc.dram_tensor("wb", [1, O], F32, kind="ExternalInput")
    out_d = nc.dram_tensor("out", [1, O], F32, kind="ExternalOutput")

    with tile.TileContext(nc) as tc:
        with (
            tc.tile_pool(name="sb", bufs=1) as sb,
            tc.tile_pool(name="ps", bufs=1, space="PSUM") as ps,
        ):
            # ---- input DMAs. gt+qwt (both needed to start the h chain)
            # ride ONE 192KB DMA on the sync queue — the per-DMA doorbell
            # latency is ~0.9us and separate queues serialize their
            # transfers anyway, so one merged DMA lands both ~0.7us sooner.
            # wwt (needed last) issues in parallel on the scalar queue. ----
            gtq = sb.tile([128, 768], BF16)
            nc.sync.dma_start(out=gtq[:], in_=gtq_d[:])
            gt = gtq  # cols 0:256 = gt
            qwt_off = 256  # cols 256:768 = qwt
            wwt = sb.tile([128, 1024], BF16)
            nc.scalar.dma_start(out=wwt[:], in_=wwt_d[:])
            if has_qb:
                qb = sb.tile([1, H], BF16)
                nc.gpsimd.dma_start(out=qb[:], in_=qb_d[:])
            if has_wb:
                wb = sb.tile([1, O], F32)
                nc.gpsimd.dma_start(out=wb[:], in_=wb_d[:])

            # ---- constants (no DMA deps) ----
            ones_m = sb.tile([K, 128], BF16)
            nc.gpsimd.memset(ones_m[:], 1.0)
            if has_qb:
                onesk = sb.tile([1, K], BF16)
                nc.gpsimd.memset(onesk[:], 1.0)

            if USE_DVE_RSQRT:
                # magic-constant tile for the rsqrt seed
                rsq_c = sb.tile([1, 1], I32)
                nc.vector.memset(rsq_c[:], 0x5F3759DF)
            else:
                # ---- warm the Sqrt ACT table: the compiler inserts each ACT
                # table load right before the first ACT that uses it, in
                # queue order. Without this warm the Sqrt table (1283ns load)
                # lands on the critical path right before the final [1,1]
                # sqrt. The warm input comes from a DVE memset so it has no
                # DMA dependency. ----
                warm_in = sb.tile([1, 1], F32)
                nc.vector.memset(warm_in[:], 1.0)
                warm_t = sb.tile([1, 1], F32)
                nc.scalar.activation(out=warm_t[:], in_=warm_in[:], func=AF.Sqrt)

            # ---- den = sum(w) broadcast across 128 partitions via
            # ones-matrix matmul (ones[K,128].T @ w = sum(w) per partition) ----
            den_bp = ps.tile([128, 1], F32, tag="rb")
            nc.tensor.matmul(
                out=den_bp[:], lhsT=ones_m[:], rhs=gt[0:K, 128:129],
                start=True, stop=True, skip_group_check=True,
            )

            # ---- h = leaky(gT.T @ Qw (+Qb)), split by h-column halves.
            # Separate PSUM tiles per half so each accumulation group gets
            # its own PSUM bank. ----
            h_ps = []
            for j in range(2):
                h_half = ps.tile([K, 128], F32, tag=f"h{j}", name=f"h_half{j}")
                h_ps.append(h_half)
            for j in range(2):
                nc.tensor.matmul(
                    out=h_ps[j][:], lhsT=gt[:, 0:64],
                    rhs=gtq[:, qwt_off + 128 * j : qwt_off + 128 * (j + 1)],
                    start=True, stop=False, skip_group_check=True,
                )
                nc.tensor.matmul(
                    out=h_ps[j][:], lhsT=gt[:, 64:128],
                    rhs=gtq[:, qwt_off + 256 + 128 * j : qwt_off + 384 + 128 * j],
                    start=False, stop=not has_qb, skip_group_check=True,
                )
                if has_qb:
                    nc.tensor.matmul(
                        out=h_ps[j][:], lhsT=onesk[:],
                        rhs=qb[:, 128 * j : 128 * (j + 1)],
                        start=False, stop=True, skip_group_check=True,
                    )

            # ---- DVE: e_scaled = e_node * den (per-partition scalar from
            # the den broadcast), then leaky-relu h halves as max(0.3x, x) ----
            e_s = sb.tile([128, 2], BF16)
            nc.vector.tensor_scalar_mul(e_s[:, 0:1], gt[:, 132:133], den_bp[:])
            nc.vector.tensor_scalar_mul(e_s[:, 1:2], gt[:, 136:137], den_bp[:])
            h_l = sb.tile([K, H], BF16)
            for j in range(2):
                nc.scalar.activation(
                    out=h_l[:, 128 * j : 128 * (j + 1)], in_=h_ps[j][:],
                    func=AF.Prelu, alpha=ALPHA,
                )

            # ---- s chunks on PE back-to-back, then the x group:
            # node parts (den-scaled) first, s parts last. High priority so
            # the scheduler runs them as soon as their leaky half is done
            # (z1 gates the last x matmuls -> the whole epilogue). ----
            s_ps = []
            for j in range(2):
                s_p = ps.tile([128, 1], F32, tag=f"s{j}", name=f"s_p{j}")
                nc.tensor.matmul(
                    out=s_p[:], lhsT=h_l[:, 128 * j : 128 * (j + 1)],
                    rhs=gt[0:K, 128:129], start=True, stop=True,
                    skip_group_check=True,
                )
                s_ps.append(s_p)

            # PSUM -> SBUF copies of the s chunks (plain, no scaling needed)
            zs = []
            for j in range(2):
                z = sb.tile([128, 1], BF16, tag=f"z{j}", name=f"z{j}")
                nc.vector.tensor_scalar_mul(z[:], s_ps[j][:], 1.0)
                zs.append(z)

            # ---- x in two [1,128] column halves: 8 cheap free=128 matmuls
            # instead of 4 free=256 ones, and the first half's epilogue
            # (leaky + square-sum) overlaps the second half's matmuls ----
            if has_wb:
                den_sb = sb.tile([1, 1], F32)
                nc.vector.tensor_scalar_mul(den_sb[:], den_bp[0:1, :], 1.0)
            o2 = sb.tile([1, O], F32)
            n2s = []
            for m in range(2):
                x_p = ps.tile([1, 128], F32, tag=f"x{m}", name=f"x{m}")
                for j in range(2):
                    nc.tensor.matmul(
                        out=x_p[:], lhsT=e_s[:, j : j + 1],
                        rhs=wwt[:, 256 * j + 128 * m : 256 * j + 128 * (m + 1)],
                        start=(j == 0), stop=False, skip_group_check=True,
                    )
                for j in range(2):
                    nc.tensor.matmul(
                        out=x_p[:], lhsT=zs[j][:],
                        rhs=wwt[:, 512 + 256 * j + 128 * m : 512 + 256 * j + 128 * (m + 1)],
                        start=False, stop=(j == 1), skip_group_check=True,
                    )
                if has_wb:
                    # x2 = wb * den + x  (bias must also be den-scaled)
                    x2 = sb.tile([1, 128], F32, name=f"x2_{m}")
                    nc.vector.scalar_tensor_tensor(
                        out=x2[:], in0=wb[:, 128 * m : 128 * (m + 1)],
                        scalar=den_sb[:], in1=x_p[:], op0=MULT, op1=ADD,
                    )
                    xsrc = x2
                else:
                    xsrc = x_p
                o2h = o2[:, 128 * m : 128 * (m + 1)]
                nc.scalar.activation(
                    out=o2h, in_=xsrc[:], func=AF.Prelu, alpha=ALPHA
                )
                sq = sb.tile([1, 128], F32, name=f"sq{m}")
                n2 = sb.tile([1, 1], F32, name=f"n2_{m}")
                nc.vector.scalar_tensor_tensor(
                    out=sq[:], in0=o2h, scalar=1.0, in1=o2h,
                    op0=MULT, op1=MULT, accum_out=n2[:],
                )
                n2s.append(n2)

            res = sb.tile([1, O], F32)
            if USE_DVE_RSQRT:
                # rc2 = rsqrt(n2a + n2b), entirely on DVE (no engine hops):
                # seed y0 = bitcast(C - (bitcast(n2) >> 1)), one Newton step
                # y1 = y0 * (1.5 - 0.5 * n2 * y0^2).
                n2 = sb.tile([1, 1], F32)
                nc.vector.scalar_tensor_tensor(
                    out=n2[:], in0=n2s[0][:], scalar=1.0, in1=n2s[1][:],
                    op0=MULT, op1=ADD,
                )
                t1 = sb.tile([1, 1], I32)
                nc.vector.tensor_scalar(
                    out=t1[:], in0=n2[:].bitcast(I32), scalar1=1, scalar2=None,
                    op0=ASR,
                )
                y0 = sb.tile([1, 1], F32)
                nc.vector.scalar_tensor_tensor(
                    out=y0[:].bitcast(I32), in0=rsq_c[:], scalar=0, in1=t1[:],
                    op0=BYP, op1=SUB,
                )
                u = sb.tile([1, 1], F32)
                nc.vector.scalar_tensor_tensor(
                    out=u[:], in0=y0[:], scalar=0.0, in1=y0[:],
                    op0=BYP, op1=MULT,
                )
                v = sb.tile([1, 1], F32)
                nc.vector.tensor_scalar(
                    out=v[:], in0=u[:], scalar1=n2[:], scalar2=-0.5,
                    op0=MULT, op1=MULT,
                )
                rc2 = sb.tile([1, 1], F32)
                nc.vector.scalar_tensor_tensor(
                    out=rc2[:], in0=v[:], scalar=1.5, in1=y0[:],
                    op0=ADD, op1=MULT,
                )
            else:
                # nrm = sqrt(n2a + n2b) in one ACT op via the bias input
                nrm = sb.tile([1, 1], F32)
                nc.scalar.activation(
                    out=nrm[:], in_=n2s[1][:], func=AF.Sqrt, bias=n2s[0][:]
                )
                rc2 = sb.tile([1, 1], F32)
                nc.vector.reciprocal(rc2[:], nrm[:])
            nc.vector.tensor_scalar_mul(res[:], o2[:], rc2[:])

            nc.sync.dma_start(out=out_d[:], in_=res[:], single_packet=True)

    nc.finalize()
    return nc


@functools.lru_cache(maxsize=4)
def _program(has_qb: bool, has_wb: bool) -> bass.Bass:
    return _build_program(has_qb, has_wb)


def kernel(
    embeddings: np.ndarray,
    weights: np.ndarray,
    Qw: np.ndarray,
    Qb: np.ndarray,
    Ww: np.ndarray,
    Wb: np.ndarray,
    neighbor_set: np.ndarray,
    node_id,
    _trace: bool = False,
):
    import ml_dtypes

    bf16 = ml_dtypes.bfloat16
    node_id = int(np.asarray(node_id))
    nbr = np.asarray(neighbor_set).astype(np.int64).reshape(K)
    emb = np.asarray(embeddings, dtype=np.float32)
    qb_full = np.asarray(Qb, dtype=np.float32).reshape(H)
    wb_full = np.asarray(Wb, dtype=np.float32).reshape(O)
    has_qb = bool(np.any(qb_full))
    has_wb = bool(np.any(wb_full))

    # shared (core-independent) weight tiles
    qw_np = np.asarray(Qw, dtype=np.float32)
    ww_np = np.asarray(Ww, dtype=np.float32)
    qwt = np.concatenate([qw_np[0:128, :], qw_np[128:256, :]], axis=1).astype(bf16)
    wwt = np.concatenate(
        [ww_np[128 * j : 128 * (j + 1), :] for j in range(4)], axis=1
    ).astype(bf16)
    wcol = np.asarray(weights[nbr, node_id], dtype=np.float32)  # [K]

    nc = _program(has_qb, has_wb)
    in_maps = []
    for b in range(N_CORES):
        g = emb[b, nbr, :]  # [K, C]
        e_node = emb[b, node_id, :]  # [C]
        gtq = np.zeros((128, 768), dtype=bf16)
        gt = np.zeros((128, 256), dtype=np.float32)
        gt[:, 0:64] = g[:, 0:128].T
        gt[:, 64:128] = g[:, 128:256].T
        gt[0:K, 128] = wcol
        gt[:, 132] = e_node[0:128]
        gt[:, 136] = e_node[128:256]
        gtq[:, 0:256] = gt.astype(bf16)
        gtq[:, 256:768] = qwt
        m = {"gtq": gtq, "wwt": wwt}
        if has_qb:
            m["qb"] = qb_full.reshape(1, H).astype(bf16)
        if has_wb:
            m["wb"] = np.ascontiguousarray(wb_full.reshape(1, O))
        in_maps.append(m)

    r = run_bass_kernel_spmd(nc, in_maps, list(range(N_CORES)), trace=_trace)
    out = np.stack([r.results[b]["out"][0] for b in range(N_CORES)], axis=0)
    if _trace:
        return out, r
    return out


# revision 29
# speedup vs baseline: 1.1528x; 1.1247x over previous
"""GNN message-passing (Convolve) kernel for Trainium2, 8 NeuronCores.

Reference computation (B=8, N=8192, C=256, H=256, O=256, K=64):
    g   = embeddings[:, neighbor_set, :]                     # [B, K, C]
    h   = leaky_relu(g @ Qw + Qb)                            # [B, K, H]
    w   = weights[neighbor_set, node_id]                     # [K]
    s   = sum_k h * w / (sum_k w + eps)                      # [B, H]
    z   = concat(embeddings[:, node_id, :], s)               # [B, C+H]
    o   = leaky_relu(z @ Ww + Wb)                            # [B, O]
    out = o / (||o||_2 + eps)                                # [B, O]

Sharding: data-parallel over the batch axis — core b handles batch b.
The host performs all *indexing/layout* work (neighbor gather, transpose,
bf16 cast, weight-column extraction); every FLOP of the reference
computation (both matmuls, the weighted sum, the activations, the L2
normalization) runs on device.

Per-core device inputs (bf16):
    gtq [128, 720]:  cols 0:64 = g[:, 0:128].T, cols 64:128 = g[:,128:256].T,
                     col 132/136 = node embedding halves,
                     row 0 cols 144:208 = w as a row,
                     cols 208:720 = [Qw[0:128, :] | Qw[128:256, :]]
    wwt [128, 1024]: [Ww[0:128,:] | Ww[128:256,:] | Ww[256:384,:] | Ww[384:512,:]]

Device dataflow (engineered around ~0.9us DMA doorbell latency, in-order
engine queues, and a long serial dependency chain of small ops):
  - gtq rides ONE DMA on the sync queue (everything the h chain needs),
    wwt (only needed by the late x matmuls) in parallel on scalar.
  - h is computed TRANSPOSED: h_T[halves m] [128, 64] = Qw_m^T @ g^T via
    4 PE matmuls with free dim 64.  That makes s a free-axis weighted
    reduction the DVE can do directly into SBUF (no PE s-matmuls, no
    PSUM->SBUF z copies): s = sum_k h_lT[:, k] * w_b[:, k] where
    w_b = ones[1,128]^T @ w_row is a tiny PE broadcast matmul.
  - den = sum(w) is a DVE reduce of w_b; 1/den folds into the leaky
    ACT's per-partition scale input (leaky is positively homogeneous).
  - x = z^T @ Ww runs as two [1,128] column halves (8 free=128 matmuls);
    the first half's epilogue (leaky + square-sum) overlaps the second
    half's matmuls; ||o||^2 = n2a + n2b is folded into the Sqrt ACT via
    its bias input.  A warm Sqrt ACT at the top hoists the table load
    off the critical path.
"""

import functools

import numpy as np

import concourse.bacc as bacc
import concourse.bass as bass
import concourse.mybir as mybir
import concourse.tile as tile
from concourse.bass_utils import run_bass_kernel_spmd

B, N, C, H, O, K = 8, 8192, 256, 256, 256, 64
ALPHA = 0.3
F32 = mybir.dt.float32
BF16 = mybir.dt.bfloat16
N_CORES = 8
MULT = mybir.AluOpType.mult
ADD = mybir.AluOpType.add
MAX = mybir.AluOpType.max
AF = mybir.ActivationFunctionType
AXX = mybir.AxisListType.X

QWT_OFF = 208  # qwt starts at this gtq column
WROW = 144  # w row 0 cols 144:208


def _build_program(has_qb: bool, has_wb: bool) -> bass.Bass:
    nc = bacc.Bacc(None, target_bir_lowering=False, debug=False)

    gtq_d = nc.dram_tensor("gtq", [128, 720], BF16, kind="ExternalInput")
    wwt_d = nc.dram_tensor("wwt", [128, 1024], BF16, kind="ExternalInput")
    if has_qb:
        qb_d = nc.dram_tensor("qb", [1, H], BF16, kind="ExternalInput")
    if has_wb:
        wb_d = nc.dram_tensor("wb", [1, O], F32, kind="ExternalInput")
    out_d = nc.dram_tensor("out", [1, O], F32, kind="ExternalOutput")

    with tile.TileContext(nc) as tc:
        with (
            tc.tile_pool(name="sb", bufs=1) as sb,
            tc.tile_pool(name="ps", bufs=1, space="PSUM") as ps,
        ):
            # ---- input DMAs ----
            gtq = sb.tile([128, 720], BF16)
            nc.sync.dma_start(out=gtq[:], in_=gtq_d[:])
            gt = gtq
            wwt = sb.tile([128, 1024], BF16)
            nc.scalar.dma_start(out=wwt[:], in_=wwt_d[:])
            if has_qb:
                qb = sb.tile([1, H], BF16)
                nc.gpsimd.dma_start(out=qb[:], in_=qb_d[:])
            if has_wb:
                wb = sb.tile([1, O], F32)
                nc.gpsimd.dma_start(out=wb[:], in_=wb_d[:])

            # ---- constants (no DMA deps) ----
            ones_r = sb.tile([1, 128], BF16)
            nc.gpsimd.memset(ones_r[:], 1.0)
            if has_qb:
                onesk = sb.tile([1, K], BF16)
                nc.gpsimd.memset(onesk[:], 1.0)

            # ---- warm the Sqrt ACT table (the compiler inserts each ACT
            # table load right before the first ACT using it, in queue
            # order; unwarmed, the 1283ns load would sit right before the
            # final sqrt) ----
            warm_in = sb.tile([1, 1], F32)
            nc.vector.memset(warm_in[:], 1.0)
            warm_t = sb.tile([1, 1], F32)
            nc.scalar.activation(out=warm_t[:], in_=warm_in[:], func=AF.Sqrt)

            # ---- w broadcast across partitions: w_b[p, k] = w[k] via a
            # tiny ones[1,128]^T @ w_row matmul; den = sum_k w on DVE ----
            w_b = ps.tile([128, K], F32, tag="wb")
            nc.tensor.matmul(
                out=w_b[:], lhsT=ones_r[:], rhs=gt[0:1, WROW : WROW + K],
                start=True, stop=True, skip_group_check=True,
            )
            den_bp = sb.tile([128, 1], F32)
            nc.vector.reduce_sum(den_bp[:], w_b[:], axis=AXX)
            rec_b = sb.tile([128, 1], F32)
            nc.vector.reciprocal(rec_b[:], den_bp[:])

            # ---- h TRANSPOSED: h_T[m] [128, 64] = Qw[:, 128m:...]^T @ g^T,
            # contracting C in 2 chunks.  leaky + 1/den scale fused in the
            # ACT (scale multiplies the input; leaky is homogeneous). ----
            h_lT = sb.tile([128, 2 * K], BF16)
            h_tps = []
            for m in range(2):
                h_t = ps.tile([128, K], F32, tag=f"h{m}", name=f"h_t{m}")
                h_tps.append(h_t)
            for m in range(2):
                nc.tensor.matmul(
                    out=h_tps[m][:],
                    lhsT=gtq[:, QWT_OFF + 128 * m : QWT_OFF + 128 * (m + 1)],
                    rhs=gt[:, 0:64],
                    start=True, stop=False, skip_group_check=True,
                )
                nc.tensor.matmul(
                    out=h_tps[m][:],
                    lhsT=gtq[:, QWT_OFF + 256 + 128 * m : QWT_OFF + 384 + 128 * m],
                    rhs=gt[:, 64:128],
                    start=False, stop=not has_qb, skip_group_check=True,
                )
                if has_qb:
                    # h_T[m][p, k] += Qb[128m + p]: qb slice as lhsT, ones row
                    nc.tensor.matmul(
                        out=h_tps[m][:], lhsT=qb[:, 128 * m : 128 * (m + 1)],
                        rhs=onesk[:], start=False, stop=True,
                        skip_group_check=True,
                    )
                nc.scalar.activation(
                    out=h_lT[:, K * m : K * (m + 1)], in_=h_tps[m][:],
                    func=AF.Prelu, alpha=ALPHA, scale=rec_b[:],
                )

            # ---- s = sum_k h_lT[:, k] * w_b[:, k]: DVE weighted reduce
            # straight into SBUF (already 1/den-scaled via the ACT) ----
            s_sc = sb.tile([128, 2 * K], F32)
            s_sb = sb.tile([128, 2], F32)
            for m in range(2):
                nc.vector.scalar_tensor_tensor(
                    out=s_sc[:, K * m : K * (m + 1)],
                    in0=h_lT[:, K * m : K * (m + 1)], scalar=1.0,
                    in1=w_b[:, 0:K], op0=MULT, op1=MULT,
                    accum_out=s_sb[:, m : m + 1],
                )
            zs = sb.tile([128, 2], BF16)
            nc.vector.tensor_scalar_mul(zs[:], s_sb[:], 1.0)

            # ---- x in two [1,128] column halves: 8 free=128 matmuls; the
            # first half's epilogue overlaps the second half's matmuls ----
            o2 = sb.tile([1, O], F32)
            n2s = []
            for m in range(2):
                x_p = ps.tile([1, 128], F32, tag=f"x{m}", name=f"x{m}")
                for j in range(2):
                    nc.tensor.matmul(
                        out=x_p[:], lhsT=gt[:, 132 + 4 * j : 133 + 4 * j],
                        rhs=wwt[:, 256 * j + 128 * m : 256 * j + 128 * (m + 1)],
                        start=(j == 0), stop=False, skip_group_check=True,
                    )
                for j in range(2):
                    nc.tensor.matmul(
                        out=x_p[:], lhsT=zs[:, j : j + 1],
                        rhs=wwt[:, 512 + 256 * j + 128 * m : 512 + 256 * j + 128 * (m + 1)],
                        start=False, stop=(j == 1), skip_group_check=True,
                    )
                if has_wb:
                    x2 = sb.tile([1, 128], F32, name=f"x2_{m}")
                    nc.vector.scalar_tensor_tensor(
                        out=x2[:], in0=wb[:, 128 * m : 128 * (m + 1)],
                        scalar=1.0, in1=x_p[:], op0=MULT, op1=ADD,
                    )
                    xsrc = x2
                else:
                    xsrc = x_p
                o2h = o2[:, 128 * m : 128 * (m + 1)]
                nc.scalar.activation(
                    out=o2h, in_=xsrc[:], func=AF.Prelu, alpha=ALPHA
                )
                sq = sb.tile([1, 128], F32, name=f"sq{m}")
                n2 = sb.tile([1, 1], F32, name=f"n2_{m}")
                nc.vector.scalar_tensor_tensor(
                    out=sq[:], in0=o2h, scalar=1.0, in1=o2h,
                    op0=MULT, op1=MULT, accum_out=n2[:],
                )
                n2s.append(n2)

            # nrm = sqrt(n2a + n2b) in one ACT op via the bias input
            nrm = sb.tile([1, 1], F32)
            nc.scalar.activation(
                out=nrm[:], in_=n2s[1][:], func=AF.Sqrt, bias=n2s[0][:]
            )
            res = sb.tile([1, O], F32)
            rc2 = sb.tile([1, 1], F32)
            nc.vector.reciprocal(rc2[:], nrm[:])
            nc.vector.tensor_scalar_mul(res[:], o2[:], rc2[:])

            nc.sync.dma_start(out=out_d[:], in_=res[:], single_packet=True)

    nc.finalize()
    return nc


@functools.lru_cache(maxsize=4)
def _program(has_qb: bool, has_wb: bool) -> bass.Bass:
    return _build_program(has_qb, has_wb)


def kernel(
    embeddings: np.ndarray,
    weights: np.ndarray,
    Qw: np.ndarray,
    Qb: np.ndarray,
    Ww: np.ndarray,
    Wb: np.ndarray,
    neighbor_set: np.ndarray,
    node_id,
    _trace: bool = False,
):
    import ml_dtypes

    bf16 = ml_dtypes.bfloat16
    node_id = int(np.asarray(node_id))
    nbr = np.asarray(neighbor_set).astype(np.int64).reshape(K)
    emb = np.asarray(embeddings, dtype=np.float32)
    qb_full = np.asarray(Qb, dtype=np.float32).reshape(H)
    wb_full = np.asarray(Wb, dtype=np.float32).reshape(O)
    has_qb = bool(np.any(qb_full))
    has_wb = bool(np.any(wb_full))

    # shared (core-independent) weight tiles
    qw_np = np.asarray(Qw, dtype=np.float32)
    ww_np = np.asarray(Ww, dtype=np.float32)
    qwt = np.concatenate([qw_np[0:128, :], qw_np[128:256, :]], axis=1).astype(bf16)
    wwt = np.concatenate(
        [ww_np[128 * j : 128 * (j + 1), :] for j in range(4)], axis=1
    ).astype(bf16)
    wcol = np.asarray(weights[nbr, node_id], dtype=np.float32)  # [K]

    nc = _program(has_qb, has_wb)
    in_maps = []
    for b in range(N_CORES):
        g = emb[b, nbr, :]  # [K, C]
        e_node = emb[b, node_id, :]  # [C]
        gtq = np.zeros((128, 720), dtype=bf16)
        gtl = np.zeros((128, QWT_OFF), dtype=np.float32)
        gtl[:, 0:64] = g[:, 0:128].T
        gtl[:, 64:128] = g[:, 128:256].T
        gtl[:, 132] = e_node[0:128]
        gtl[:, 136] = e_node[128:256]
        gtl[0, WROW : WROW + K] = wcol
        gtq[:, 0:QWT_OFF] = gtl.astype(bf16)
        gtq[:, QWT_OFF:720] = qwt
        m = {"gtq": gtq, "wwt": wwt}
        if has_qb:
            m["qb"] = qb_full.reshape(1, H).astype(bf16)
        if has_wb:
            m["wb"] = np.ascontiguousarray(wb_full.reshape(1, O))
        in_maps.append(m)

    r = run_bass_kernel_spmd(nc, in_maps, list(range(N_CORES)), trace=_trace)
    out = np.stack([r.results[b]["out"][0] for b in range(N_CORES)], axis=0)
    if _trace:
        return out, r
    return out


# revision 30
# speedup vs baseline: 1.2503x; 1.0846x over previous
"""GNN message-passing (Convolve) kernel for Trainium2, 8 NeuronCores.

Reference computation (B=8, N=8192, C=256, H=256, O=256, K=64):
    g   = embeddings[:, neighbor_set, :]                     # [B, K, C]
    h   = leaky_relu(g @ Qw + Qb)                            # [B, K, H]
    w   = weights[neighbor_set, node_id]                     # [K]
    s   = sum_k h * w / (sum_k w + eps)                      # [B, H]
    z   = concat(embeddings[:, node_id, :], s)               # [B, C+H]
    o   = leaky_relu(z @ Ww + Wb)                            # [B, O]
    out = o / (||o||_2 + eps)                                # [B, O]

Sharding: data-parallel over the batch axis — core b handles batch b.
The host performs all *indexing/layout* work (neighbor gather, transpose,
bf16 cast, weight-column extraction); every FLOP of the reference
computation (both matmuls, the weighted sum, the activations, the L2
normalization) runs on device.

Per-core device inputs (bf16):
    gtq [128, 720]:  cols 0:64 = g[:, 0:128].T, cols 64:128 = g[:,128:256].T,
                     col 132/136 = node embedding halves,
                     row 0 cols 144:208 = w as a row,
                     cols 208:720 = [Qw[0:128, :] | Qw[128:256, :]]
    wwt [128, 1024]: [Ww[0:128,:] | Ww[128:256,:] | Ww[256:384,:] | Ww[384:512,:]]

Device dataflow (engineered around ~0.9us DMA doorbell latency, in-order
engine queues, and a long serial dependency chain of small ops):
  - gtq rides ONE DMA on the sync queue (everything the h chain needs),
    wwt (only needed by the late x matmuls) in parallel on scalar.
  - h is computed TRANSPOSED: h_T[halves m] [128, 64] = Qw_m^T @ g^T via
    4 PE matmuls with free dim 64.  That makes s a free-axis weighted
    reduction the DVE can do directly into SBUF (no PE s-matmuls, no
    PSUM->SBUF z copies): s = sum_k h_lT[:, k] * w_b[:, k] where
    w_b = ones[1,128]^T @ w_row is a tiny PE broadcast matmul.
  - den = sum(w) is a DVE reduce of w_b; 1/den folds into the leaky
    ACT's per-partition scale input (leaky is positively homogeneous).
  - x = z^T @ Ww runs as two [1,128] column halves (8 free=128 matmuls);
    the first half's epilogue (leaky + square-sum) overlaps the second
    half's matmuls; ||o||^2 = n2a + n2b is folded into the Sqrt ACT via
    its bias input.  A warm Sqrt ACT at the top hoists the table load
    off the critical path.
"""

import functools

import numpy as np

import concourse.bacc as bacc
import concourse.bass as bass
import concourse.mybir as mybir
import concourse.tile as tile
from concourse.bass_utils import run_bass_kernel_spmd

B, N, C, H, O, K = 8, 8192, 256, 256, 256, 64
ALPHA = 0.3
F32 = mybir.dt.float32
BF16 = mybir.dt.bfloat16
N_CORES = 8
MULT = mybir.AluOpType.mult
ADD = mybir.AluOpType.add
MAX = mybir.AluOpType.max
AF = mybir.ActivationFunctionType
AXX = mybir.AxisListType.X

QWT_OFF = 208  # qwt starts at this gtq column
WROW = 144  # w row 0 cols 144:208


def _build_program(has_qb: bool, has_wb: bool) -> bass.Bass:
    nc = bacc.Bacc(None, target_bir_lowering=False, debug=False)

    gtq_d = nc.dram_tensor("gtq", [128, 720], BF16, kind="ExternalInput")
    wwt_d = nc.dram_tensor("wwt", [128, 1024], BF16, kind="ExternalInput")
    if has_qb:
        qb_d = nc.dram_tensor("qb", [1, H], BF16, kind="ExternalInput")
    if has_wb:
        wb_d = nc.dram_tensor("wb", [1, O], F32, kind="ExternalInput")
    out_d = nc.dram_tensor("out", [1, O], F32, kind="ExternalOutput")

    with tile.TileContext(nc) as tc:
        with (
            tc.tile_pool(name="sb", bufs=1) as sb,
            tc.tile_pool(name="ps", bufs=1, space="PSUM") as ps,
        ):
            # ---- input DMAs ----
            gtq = sb.tile([128, 720], BF16)
            nc.sync.dma_start(out=gtq[:], in_=gtq_d[:])
            gt = gtq
            wwt = sb.tile([128, 1024], BF16)
            nc.scalar.dma_start(out=wwt[:], in_=wwt_d[:])
            if has_qb:
                qb = sb.tile([1, H], BF16)
                nc.gpsimd.dma_start(out=qb[:], in_=qb_d[:])
            if has_wb:
                wb = sb.tile([1, O], F32)
                nc.gpsimd.dma_start(out=wb[:], in_=wb_d[:])

            # ---- constants (no DMA deps) ----
            ones_r = sb.tile([1, 128], BF16)
            nc.gpsimd.memset(ones_r[:], 1.0)
            if has_qb:
                onesk = sb.tile([1, K], BF16)
                nc.gpsimd.memset(onesk[:], 1.0)

            # ---- warm the Sqrt ACT table (the compiler inserts each ACT
            # table load right before the first ACT using it, in queue
            # order; unwarmed, the 1283ns load would sit right before the
            # final sqrt) ----
            warm_in = sb.tile([1, 1], F32)
            nc.vector.memset(warm_in[:], 1.0)
            warm_t = sb.tile([1, 1], F32)
            nc.scalar.activation(out=warm_t[:], in_=warm_in[:], func=AF.Sqrt)

            # ---- w broadcast across partitions: w_b[p, k] = w[k] via a
            # tiny ones[1,128]^T @ w_row matmul; den = sum_k w on DVE ----
            w_b = ps.tile([128, K], F32, tag="wb")
            nc.tensor.matmul(
                out=w_b[:], lhsT=ones_r[:], rhs=gt[0:1, WROW : WROW + K],
                start=True, stop=True, skip_group_check=True,
            )
            den_bp = sb.tile([128, 1], F32)
            nc.vector.reduce_sum(den_bp[:], w_b[:], axis=AXX)
            rec_b = sb.tile([128, 1], F32)
            nc.vector.reciprocal(rec_b[:], den_bp[:])

            # ---- h TRANSPOSED: h_T[m] [128, 64] = Qw[:, 128m:...]^T @ g^T,
            # contracting C in 2 chunks.  leaky + 1/den scale fused in the
            # ACT (scale multiplies the input; leaky is homogeneous). ----
            h_lT = sb.tile([128, 2 * K], BF16)
            h_tps = []
            for m in range(2):
                h_t = ps.tile([128, K], F32, tag=f"h{m}", name=f"h_t{m}")
                h_tps.append(h_t)
            for m in range(2):
                nc.tensor.matmul(
                    out=h_tps[m][:],
                    lhsT=gtq[:, QWT_OFF + 128 * m : QWT_OFF + 128 * (m + 1)],
                    rhs=gt[:, 0:64],
                    start=True, stop=False, skip_group_check=True,
                )
                nc.tensor.matmul(
                    out=h_tps[m][:],
                    lhsT=gtq[:, QWT_OFF + 256 + 128 * m : QWT_OFF + 384 + 128 * m],
                    rhs=gt[:, 64:128],
                    start=False, stop=not has_qb, skip_group_check=True,
                )
                if has_qb:
                    # h_T[m][p, k] += Qb[128m + p]: qb slice as lhsT, ones row
                    nc.tensor.matmul(
                        out=h_tps[m][:], lhsT=qb[:, 128 * m : 128 * (m + 1)],
                        rhs=onesk[:], start=False, stop=True,
                        skip_group_check=True,
                    )
                nc.scalar.activation(
                    out=h_lT[:, K * m : K * (m + 1)], in_=h_tps[m][:],
                    func=AF.Prelu, alpha=ALPHA,
                )

            # ---- s = sum_k (h_lT[:, k] / den) * w_b[:, k]: DVE weighted
            # reduce straight into SBUF; the 1/den scale rides the STT's
            # otherwise-unused scalar slot (leaky is homogeneous, so
            # scaling after the activation is equivalent).  Per-half z
            # casts so each xz matmul unblocks as soon as its half is
            # reduced. ----
            s_sc = sb.tile([128, 2 * K], F32)
            s_sb = sb.tile([128, 2], F32)
            zs = sb.tile([128, 2], BF16)
            for m in range(2):
                nc.vector.scalar_tensor_tensor(
                    out=s_sc[:, K * m : K * (m + 1)],
                    in0=h_lT[:, K * m : K * (m + 1)], scalar=rec_b[:],
                    in1=w_b[:, 0:K], op0=MULT, op1=MULT,
                    accum_out=s_sb[:, m : m + 1],
                )
                nc.vector.tensor_scalar_mul(
                    zs[:, m : m + 1], s_sb[:, m : m + 1], 1.0
                )

            # ---- x in two [1,128] column halves: 8 free=128 matmuls; the
            # first half's epilogue overlaps the second half's matmuls ----
            o2 = sb.tile([1, O], F32)
            n2s = []
            for m in range(2):
                x_p = ps.tile([1, 128], F32, tag=f"x{m}", name=f"x{m}")
                for j in range(2):
                    nc.tensor.matmul(
                        out=x_p[:], lhsT=gt[:, 132 + 4 * j : 133 + 4 * j],
                        rhs=wwt[:, 256 * j + 128 * m : 256 * j + 128 * (m + 1)],
                        start=(j == 0), stop=False, skip_group_check=True,
                    )
                for j in range(2):
                    nc.tensor.matmul(
                        out=x_p[:], lhsT=zs[:, j : j + 1],
                        rhs=wwt[:, 512 + 256 * j + 128 * m : 512 + 256 * j + 128 * (m + 1)],
                        start=False, stop=(j == 1), skip_group_check=True,
                    )
                if has_wb:
                    x2 = sb.tile([1, 128], F32, name=f"x2_{m}")
                    nc.vector.scalar_tensor_tensor(
                        out=x2[:], in0=wb[:, 128 * m : 128 * (m + 1)],
                        scalar=1.0, in1=x_p[:], op0=MULT, op1=ADD,
                    )
                    xsrc = x2
                else:
                    xsrc = x_p
                o2h = o2[:, 128 * m : 128 * (m + 1)]
                nc.scalar.activation(
                    out=o2h, in_=xsrc[:], func=AF.Prelu, alpha=ALPHA
                )
                sq = sb.tile([1, 128], F32, name=f"sq{m}")
                n2 = sb.tile([1, 1], F32, name=f"n2_{m}")
                nc.vector.scalar_tensor_tensor(
                    out=sq[:], in0=o2h, scalar=1.0, in1=o2h,
                    op0=MULT, op1=MULT, accum_out=n2[:],
                )
                n2s.append(n2)

            # nrm = sqrt(n2a + n2b) in one ACT op via the bias input
            nrm = sb.tile([1, 1], F32)
            nc.scalar.activation(
                out=nrm[:], in_=n2s[1][:], func=AF.Sqrt, bias=n2s[0][:]
            )
            res = sb.tile([1, O], F32)
            rc2 = sb.tile([1, 1], F32)
            nc.vector.reciprocal(rc2[:], nrm[:])
            nc.vector.tensor_scalar_mul(res[:], o2[:], rc2[:])

            nc.sync.dma_start(out=out_d[:], in_=res[:], single_packet=True)

    nc.finalize()
    return nc


@functools.lru_cache(maxsize=4)
def _program(has_qb: bool, has_wb: bool) -> bass.Bass:
    return _build_program(has_qb, has_wb)


def kernel(
    embeddings: np.ndarray,
    weights: np.ndarray,
    Qw: np.ndarray,
    Qb: np.ndarray,
    Ww: np.ndarray,
    Wb: np.ndarray,
    neighbor_set: np.ndarray,
    node_id,
    _trace: bool = False,
):
    import ml_dtypes

    bf16 = ml_dtypes.bfloat16
    node_id = int(np.asarray(node_id))
    nbr = np.asarray(neighbor_set).astype(np.int64).reshape(K)
    emb = np.asarray(embeddings, dtype=np.float32)
    qb_full = np.asarray(Qb, dtype=np.float32).reshape(H)
    wb_full = np.asarray(Wb, dtype=np.float32).reshape(O)
    has_qb = bool(np.any(qb_full))
    has_wb = bool(np.any(wb_full))

    # shared (core-independent) weight tiles
    qw_np = np.asarray(Qw, dtype=np.float32)
    ww_np = np.asarray(Ww, dtype=np.float32)
    qwt = np.concatenate([qw_np[0:128, :], qw_np[128:256, :]], axis=1).astype(bf16)
    wwt = np.concatenate(
        [ww_np[128 * j : 128 * (j + 1), :] for j in range(4)], axis=1
    ).astype(bf16)
    wcol = np.asarray(weights[nbr, node_id], dtype=np.float32)  # [K]

    nc = _program(has_qb, has_wb)
    in_maps = []
    for b in range(N_CORES):
        g = emb[b, nbr, :]  # [K, C]
        e_node = emb[b, node_id, :]  # [C]
        gtq = np.zeros((128, 720), dtype=bf16)
        gtl = np.zeros((128, QWT_OFF), dtype=np.float32)
        gtl[:, 0:64] = g[:, 0:128].T
        gtl[:, 64:128] = g[:, 128:256].T
        gtl[:, 132] = e_node[0:128]
        gtl[:, 136] = e_node[128:256]
        gtl[0, WROW : WROW + K] = wcol
        gtq[:, 0:QWT_OFF] = gtl.astype(bf16)
        gtq[:, QWT_OFF:720] = qwt
        m = {"gtq": gtq, "wwt": wwt}
        if has_qb:
            m["qb"] = qb_full.reshape(1, H).astype(bf16)
        if has_wb:
            m["wb"] = np.ascontiguousarray(wb_full.reshape(1, O))
        in_maps.append(m)

    r = run_bass_kernel_spmd(nc, in_maps, list(range(N_CORES)), trace=_trace)
    out = np.stack([r.results[b]["out"][0] for b in range(N_CORES)], axis=0)
    if _trace:
        return out, r
    return out
